# revision 25
# baseline (speedup 1.0000x reference)
"""Trainium2 Bass kernel for nn_CAMLocalHead (CAM target + conv head + BCE).

Self-contained: takes FULL inputs, shards batch B=8 across 8 NeuronCores
(one sample per core), runs a Bass/Tile kernel per core, combines on host.

Device does the 99.99%-of-FLOPs part: Conv3d(2048->512, 1x3x3, pad 011)
+ ReLU + score Conv3d(512->1), via 1D-Winograd F(2,3) along the w axis in
fp8 DoubleRow matmuls (384 MMs of N=448 vs 576 of N=392 direct).

Dataflow is organized for FLAT DMA demand (the v1 layout front-loaded
~11MB into the first 20us, starving the PE and tripping the HAM clock
gate down to 1.2 GHz): the Winograd point index `pt` is the OUTERMOST
loop, and weights U[pt] / inputs xt[pt] stream just-in-time per (pt,ctp)
at a sustained ~215 GB/s.  Each pass accumulates 4 d-tile psums over
(ctp 8 x dy 3); at pass end the psums fold incrementally into SBUF
o0/o1 accumulators with A^T signs (o0 = p0+p1+p2, o1 = p1-p2-p3), so
only one PSUM operand per DVE op and only 4 psum banks live per pass.

Host does the CAM target branch (argmax class, proj row, top-392 mask --
0.02% of FLOPs, exact fp32 like the reference) and the final BCE scalar
reduce over 784 logits/sample.
"""
import sys

for _p in ("/opt/trn_rl_repo", "/opt/pypackages"):
    if _p not in sys.path:
        sys.path.append(_p)

import numpy as np
import ml_dtypes

# Problem dims (hardcoded per spec)
B, C, T, H, W = 8, 2048, 16, 7, 7
K, D = 400, 512
N_TOKEN = 392
P = 128
CT = C // P          # 16 c-tiles
CTP = CT // 2        # 8 c-tile pairs (DoubleRow)
DT = D // P          # 4 d-tiles
YP = 9               # padded height (1+7+1)
NJ = 4               # width tiles of 2 outputs (covers padded width 8)
NPT = 4              # Winograd points F(2,3)
NF = T * H * NJ      # 448 free-dim per matmul (t, y, j)
NF3 = T * H * 3      # 336: odd-x side needs only j 0..2 (x=7 is pad)
PADO = NF + NF3      # 784 output row: [even-x 448 | odd-x 336]
PADN = 2 * NF        # 896 = padded-width-8 (host-side naming only)
XCF = 2 * T * YP * NJ         # 1152: free size of one (pt,ctp) x chunk
WSF = DT * 3 * 2 * P          # 3072: free size of one (pt,ctp) w slab

_cache = {}


def _build_nc():
    import concourse.bacc as bacc
    import concourse.mybir as mybir
    from concourse import tile

    f32 = mybir.dt.float32
    bf16 = mybir.dt.bfloat16
    fp8 = mybir.dt.float8e4
    DR = mybir.MatmulPerfMode.DoubleRow
    OP = mybir.AluOpType
    AF = mybir.ActivationFunctionType

    nc = bacc.Bacc(trn_type="TRN2")

    u8_d = nc.dram_tensor("u8", [NPT, P, CTP * WSF], fp8,
                          kind="ExternalInput")
    xt8_d = nc.dram_tensor("xt8", [NPT * CTP, P, XCF], fp8,
                           kind="ExternalInput")
    cb_d = nc.dram_tensor("cb", [P, DT], f32, kind="ExternalInput")
    sw_d = nc.dram_tensor("sw", [P, 32], fp8, kind="ExternalInput")
    out_d = nc.dram_tensor("out", [1, PADO], f32, kind="ExternalOutput")

    with tile.TileContext(nc) as tc:
        with (
            tc.tile_pool(name="const", bufs=1) as cp,
            tc.tile_pool(name="wps_", bufs=6) as wp,
            tc.tile_pool(name="xps_", bufs=10) as xp,
            tc.tile_pool(name="rp", bufs=4) as rp,
            tc.tile_pool(name="cps", bufs=6, space="PSUM") as cps,
            tc.tile_pool(name="sps", bufs=1, space="PSUM") as sps,
        ):
            # PE warm-up first: memsets on GpSimd (free ~1us before Vector
            # in the framework preamble), then dummy bf16 matmuls with no
            # DMA deps run during the DMA lead-in, flipping the HAM clock
            # gate to 8/8 before the real conv stream starts.
            wrm_in = cp.tile([P, 2 * P], bf16)
            nc.gpsimd.memset(wrm_in[:], 0.0)
            ones_cb = cp.tile([P, 1], bf16)
            nc.gpsimd.memset(ones_cb[:], 1.0)
            for i in range(11):
                wrm_ps = cps.tile([1, 2 * P], f32, tag="cv", name=f"wrm{i}")
                nc.tensor.matmul(wrm_ps[:], ones_cb[:], wrm_in[:],
                                 start=True, stop=True)

            # first (pt0,ctp0) tiles: xt0 on the scalar ring runs in
            # parallel with the w chunks on the sync ring, so the first
            # conv matmul's data lands as the warm-up matmuls finish
            xt0 = xp.tile([P, XCF], fp8, name="xt", tag="xt")
            nc.scalar.dma_start(xt0[:], xt8_d[0])
            xt1 = xp.tile([P, XCF], fp8, name="xt", tag="xt")
            nc.scalar.dma_start(xt1[:], xt8_d[1])
            w_dts0 = []
            for dt in range(DT):
                w_dt = wp.tile([P, 6 * P], fp8, name="w_dt", tag="w_sl")
                nc.sync.dma_start(
                    w_dt[:], u8_d[0][:, dt * 6 * P:(dt + 1) * 6 * P])
                w_dts0.append(w_dt)

            # ---------- small constants (scalar HWDGE ring) ----------
            cb_sb = cp.tile([P, DT], f32)
            nc.scalar.dma_start(cb_sb[:], cb_d[:])
            # sw8[p, two*16 + g]: DR lhsT interleave stride must be %16
            sw_sb = cp.tile([P, 32], fp8)
            nc.scalar.dma_start(sw_sb[:], sw_d[:])

            # SBUF accumulators for the A^T fold, per d-tile
            o0 = [cp.tile([P, NF], f32, name=f"o0_{dt}") for dt in range(DT)]
            o1 = [cp.tile([P, NF3], f32, name=f"o1_{dt}")
                  for dt in range(DT)]
            relu_a = [cp.tile([P, 2 * NF], fp8, name=f"ra_{g}")
                      for g in range(2)]
            relu_b = [cp.tile([P, 2 * NF3], fp8, name=f"rb_{g}")
                      for g in range(2)]

            # score psums [1, 448] x2 accumulate logits across all dt
            s_ps = [sps.tile([1, NF if h == 0 else NF3], f32,
                             tag=f"s{h}", name=f"s_ps{h}")
                    for h in range(2)]

            # ---------- conv main: pt-outer passes, JIT streaming ------
            # pass 3 feeds only o1 (odd-x outputs); width-tile j=3's odd
            # output is the x=7 pad column, so pass 3 streams j 0..2 only.
            for pt in range(NPT):
                nfp = NF3 if pt == 3 else NF
                njp = 3 if pt == 3 else NJ
                xcf = 2 * T * YP * njp
                ps = [cps.tile([P, nfp], f32, tag="cv", name=f"ps{pt}_{dt}")
                      for dt in range(DT)]
                for ctp in range(CTP):
                    if pt == 3 and ctp == CTP - 1:
                        # score-A matmuls are data-ready (relu_a finished
                        # during this pass); slotting them here keeps them
                        # off the tail critical path
                        swv = sw_sb[:].rearrange(
                            "p (two x) -> p two x", two=2)
                        for g in range(2):
                            nc.tensor.matmul(
                                s_ps[0][:], swv[:, :, g:g + 1],
                                relu_a[g][:].rearrange(
                                    "p (two f) -> p two f", two=2),
                                start=(g == 0), stop=(g == 1),
                                perf_mode=DR)
                    w_dts = None
                    if pt == 0 and ctp == 0:
                        xt = xt0
                        w_sl = None
                        w_dts = w_dts0
                    else:
                        if pt == 0 and ctp == 1:
                            xt = xt1
                        else:
                            xt = xp.tile([P, xcf], fp8, name="xt",
                                         tag="xt")
                            nc.sync.dma_start(
                                xt[:], xt8_d[pt * CTP + ctp][:, 0:xcf])
                        w_sl = wp.tile([P, WSF], fp8, name="w_sl",
                                       tag="w_sl")
                        nc.sync.dma_start(
                            w_sl[:], u8_d[pt][:, ctp * WSF:(ctp + 1) * WSF])
                    xv = xt[:].rearrange("p (two t y j) -> p two t y j",
                                         two=2, t=T, y=YP)
                    dts = (list(reversed(range(DT)))
                           if pt == 3 and ctp == CTP - 1 else range(DT))
                    for dt in dts:
                        for dy in range(3):
                            if w_sl is None:
                                wsl2 = w_dts[dt][:, dy * 2 * P:
                                                 (dy + 1) * 2 * P]
                            else:
                                off = (dt * 3 + dy) * 2 * P
                                wsl2 = w_sl[:, off:off + 2 * P]
                            lhsT3 = wsl2.rearrange("p (two q) -> p two q",
                                                   two=2)
                            rhs = xv[:, :, :, dy:dy + H, :]
                            nc.tensor.matmul(
                                ps[dt][:], lhsT3, rhs,
                                start=(ctp == 0 and dy == 0),
                                stop=(ctp == CTP - 1 and dy == 2),
                                perf_mode=DR)
                # fold pass into o0/o1 (A^T signs); one PSUM operand per op
                for dt in range(DT):
                    if pt == 0:
                        nc.vector.tensor_copy(o0[dt][:], ps[dt][:])
                    elif pt == 1:
                        nc.vector.tensor_tensor(o0[dt][:], o0[dt][:],
                                                ps[dt][:], op=OP.add)
                        pv = ps[dt][:].rearrange(
                            "p (ty j) -> p ty j", j=NJ)[:, :, 0:3]
                        nc.vector.tensor_copy(o1[dt][:], pv)
                    elif pt == 2:
                        nc.vector.tensor_tensor(o0[dt][:], o0[dt][:],
                                                ps[dt][:], op=OP.add)
                        pv = ps[dt][:].rearrange(
                            "p (ty j) -> p ty j", j=NJ)[:, :, 0:3]
                        nc.vector.tensor_tensor(o1[dt][:], o1[dt][:],
                                                pv, op=OP.subtract)
                    else:
                        nc.vector.tensor_tensor(o1[dt][:], o1[dt][:],
                                                ps[dt][:], op=OP.subtract)
                if pt == 2:
                    # o0 is final after pass 2 -- its ReLUs (ACT) overlap
                    # pass 3's conv matmuls, so only the o1 side is tail.
                    for dt in range(DT):
                        dst = relu_a[dt // 2][:, (dt % 2) * NF:
                                              (dt % 2 + 1) * NF]
                        nc.scalar.activation(dst, o0[dt][:], AF.Relu,
                                             bias=cb_sb[:, dt:dt + 1],
                                             scale=1.0 / 64.0)

            # ---------- o1 ReLU + score + ship logits ----------
            xrow = cp.tile([1, PADO], f32)
            nc.vector.tensor_copy(xrow[0:1, 0:NF], s_ps[0][:])
            nc.sync.dma_start(out_d[:, 0:NF], xrow[0:1, 0:NF])
            swv2 = sw_sb[:].rearrange(
                "p (two x) -> p two x", two=2)
            rtmp = [cp.tile([P, NF3], f32, name=f"rtmp{i}")
                    for i in range(2)]
            for dt in (3, 2, 1, 0):
                dst = relu_b[dt // 2][:, (dt % 2) * NF3:(dt % 2 + 1) * NF3]
                if dt % 2 == 1:
                    nc.scalar.activation(dst, o1[dt][:], AF.Relu,
                                         bias=cb_sb[:, dt:dt + 1],
                                         scale=1.0 / 64.0)
                else:
                    # even d-tiles on DVE so the four o1-ReLUs run two
                    # engines wide instead of serializing on ACT
                    tmp = rtmp[dt // 2]
                    nc.vector.tensor_scalar(tmp[:], o1[dt][:], 1.0 / 64.0,
                                            cb_sb[:, dt:dt + 1],
                                            op0=OP.mult, op1=OP.add)
                    nc.vector.tensor_scalar(dst, tmp[:], 0.0, None,
                                            op0=OP.max)
                if dt % 2 == 0:
                    g = dt // 2
                    nc.tensor.matmul(
                        s_ps[1][:], swv2[:, :, g:g + 1],
                        relu_b[g][:].rearrange(
                            "p (two f) -> p two f", two=2),
                        start=(g == 1), stop=(g == 0),
                        perf_mode=DR)
            nc.vector.tensor_copy(xrow[0:1, NF:PADO], s_ps[1][:])
            nc.sync.dma_start(out_d[:, NF:PADO], xrow[0:1, NF:PADO])

    nc.compile()
    return nc


def _prep_in_maps(x, x_fpv_pred, proj_weight, conv1_w, conv1_b, score_w,
                  score_b):
    import concourse.mybir as mybir
    bf16 = ml_dtypes.bfloat16
    fp8 = mybir.dt.np(mybir.dt.float8e4)

    # x_tilde: pad (y: 1+7+1, x: 1+7+2), B^T along x per 2-wide tile
    xr = np.asarray(x, np.float32).reshape(B, CT, P, T, H, W)
    xpad = np.zeros((B, CT, P, T, YP, 10), np.float32)
    xpad[:, :, :, :, 1:8, 1:8] = xr
    # windows d[k] = xpad[..., 2j+k], k=0..3, j=0..3
    dw = np.stack([xpad[..., 2 * j:2 * j + 4] for j in range(NJ)], axis=-2)
    # dw: [B, CT, P, T, YP, NJ, 4]
    xt = np.empty((B, NPT, CT, P, T, YP, NJ), np.float32)
    xt[:, 0] = dw[..., 0] - dw[..., 2]
    xt[:, 1] = dw[..., 1] + dw[..., 2]
    xt[:, 2] = dw[..., 2] - dw[..., 1]
    xt[:, 3] = dw[..., 1] - dw[..., 3]
    # device layout [B, (pt, ctp), P, (two, t, y, j)]; pt3 packs j 0..2
    xt = xt.reshape(B, NPT, CTP, 2, P, T, YP, NJ)
    xtt = xt.transpose(0, 1, 2, 4, 3, 5, 6, 7)  # [B,pt,ctp,P,two,T,YP,NJ]
    xt8 = np.zeros((B, NPT * CTP, P, XCF), fp8)
    xt8[:, :3 * CTP] = xtt[:, :3].reshape(B, 3 * CTP, P, XCF).astype(fp8)
    xt8[:, 3 * CTP:, :, :2 * T * YP * 3] = np.ascontiguousarray(
        xtt[:, 3, :, :, :, :, :, 0:3]).reshape(
        B, CTP, P, 2 * T * YP * 3).astype(fp8)

    # U = G @ w along dx, x64, fp8.  G rows: [1,0,0],[.5,.5,.5],
    # [.5,-.5,.5],[0,0,1]
    w9 = np.asarray(conv1_w, np.float32).reshape(D, C, 3, 3)
    G = np.array([[1, 0, 0], [.5, .5, .5], [.5, -.5, .5], [0, 0, 1]],
                 np.float32)
    u = np.einsum('pk,dcyk->dcyp', G, w9) * 64.0   # [D, C, 3dy, NPT]
    # layout u8[pt, p, ((ctp*DT + dt)*3 + dy)*2*P + two*P + q]
    #   = u[dt*P+q, (2*ctp+two)*P+p, dy, pt]
    u8 = np.ascontiguousarray(
        u.reshape(DT, P, CTP, 2, P, 3, NPT)
        .transpose(6, 4, 2, 0, 5, 3, 1).reshape(NPT, P, CTP * WSF)
    ).astype(fp8)

    cb = np.ascontiguousarray(
        np.asarray(conv1_b, np.float32).reshape(DT, P).T)
    # sw8[p, two*16 + g] = 64 * score_w[(2g+two)*P + p] (stride-16 pairs)
    swr = (np.asarray(score_w, np.float32).reshape(2, 2, P) * 64.0)
    sw = np.zeros((P, 32), np.float32)
    for g in range(2):
        for two in range(2):
            sw[:, two * 16 + g] = swr[g, two]
    sw = np.ascontiguousarray(sw).astype(fp8)

    in_maps = []
    for b in range(B):
        in_maps.append({
            "xt8": xt8[b],
            "u8": u8,
            "cb": cb,
            "sw": sw,
        })
    return in_maps


def _host_cam_target(x, x_fpv_pred, proj_weight):
    """Exact fp32 CAM target (reference semantics), vectorized numpy."""
    xf = np.asarray(x, np.float32).reshape(B, C, -1)
    top_cls = np.argmax(np.asarray(x_fpv_pred, np.float32), axis=1)
    rows = np.asarray(proj_weight, np.float32)[top_cls]       # [B, C]
    cam = np.einsum('bc,bcn->bn', rows, xf)                   # [B, 784]
    cmin = cam.min(axis=1, keepdims=True)
    cmax = cam.max(axis=1, keepdims=True)
    cam_n = (cam - cmin) / (cmax - cmin)
    y = np.zeros_like(cam_n)
    idx = np.argpartition(-cam_n, N_TOKEN - 1, axis=1)[:, :N_TOKEN]
    np.put_along_axis(y, idx, np.take_along_axis(cam_n, idx, axis=1), axis=1)
    return y                                                   # [B, 784]


def run(inputs, trace=False):
    """Build (cached), run on 8 cores, return (loss, BassKernelResults)."""
    from concourse.bass_utils import run_bass_kernel_spmd

    if "nc" not in _cache:
        _cache["nc"] = _build_nc()
    nc = _cache["nc"]
    in_maps = _prep_in_maps(**inputs)
    res = run_bass_kernel_spmd(nc, in_maps, core_ids=list(range(B)),
                               trace=trace)
    y = _host_cam_target(inputs["x"], inputs["x_fpv_pred"],
                         inputs["proj_weight"])
    sb = float(np.asarray(inputs["score_b"], np.float32).reshape(-1)[0])
    total = 0.0
    for b in range(B):
        row = np.asarray(res.results[b]["out"], np.float32).reshape(PADO)
        x8 = np.empty((T * H, W), np.float32)                  # x = 2j+k
        x8[:, 0::2] = row[:NF].reshape(T * H, NJ)              # x 0,2,4,6
        x8[:, 1::2] = row[NF:].reshape(T * H, 3)               # x 1,3,5
        xcam = x8.reshape(-1) / 64.0 + sb
        # bce_sum = sum log(1+e^x) - sum x*y  (softplus stable form)
        sp = np.logaddexp(0.0, xcam).sum()
        total += float(sp - (xcam * y[b]).sum())
    loss = np.float32(total / float(B * T * H * W))
    return loss, res


def kernel(**inputs):
    loss, _ = run(inputs, trace=False)
    return loss


# revision 26
# speedup vs baseline: 1.2001x; 1.2001x over previous
"""Trainium2 Bass kernel for nn_CAMLocalHead (CAM target + conv head + BCE).

Self-contained: takes FULL inputs, shards batch B=8 across 8 NeuronCores
(one sample per core), runs a Bass/Tile kernel per core, combines on host.

Device does the 99.99%-of-FLOPs part: Conv3d(2048->512, 1x3x3, pad 011)
+ ReLU + score Conv3d(512->1), via 1D-Winograd F(2,3) along the w axis in
fp8 DoubleRow matmuls (384 MMs of N=448 vs 576 of N=392 direct).

Dataflow is organized for FLAT DMA demand (the v1 layout front-loaded
~11MB into the first 20us, starving the PE and tripping the HAM clock
gate down to 1.2 GHz): the Winograd point index `pt` is the OUTERMOST
loop, and weights U[pt] / inputs xt[pt] stream just-in-time per (pt,ctp)
at a sustained ~215 GB/s.  Each pass accumulates 4 d-tile psums over
(ctp 8 x dy 3); at pass end the psums fold incrementally into SBUF
o0/o1 accumulators with A^T signs (o0 = p0+p1+p2, o1 = p1-p2-p3), so
only one PSUM operand per DVE op and only 4 psum banks live per pass.

Host does the CAM target branch (argmax class, proj row, top-392 mask --
0.02% of FLOPs, exact fp32 like the reference) and the final BCE scalar
reduce over 784 logits/sample.
"""
import sys

for _p in ("/opt/trn_rl_repo", "/opt/pypackages"):
    if _p not in sys.path:
        sys.path.append(_p)

import numpy as np
import ml_dtypes

# Problem dims (hardcoded per spec)
B, C, T, H, W = 8, 2048, 16, 7, 7
K, D = 400, 512
N_TOKEN = 392
P = 128
CT = C // P          # 16 c-tiles
CTP = CT // 2        # 8 c-tile pairs (DoubleRow)
DT = D // P          # 4 d-tiles
YP = 9               # padded height (1+7+1)
NJ = 4               # width tiles of 2 outputs (covers padded width 8)
NPT = 4              # Winograd points F(2,3)
NF = T * H * NJ      # 448 free-dim per matmul (t, y, j)
NF3 = T * H * 3      # 336: odd-x side needs only j 0..2 (x=7 is pad)
PADO = NF + NF3      # 784 output row: [even-x 448 | odd-x 336]
PADN = 2 * NF        # 896 = padded-width-8 (host-side naming only)
XCF = 2 * T * YP * NJ         # 1152: free size of one (pt,ctp) x chunk
WSF = DT * 3 * 2 * P          # 3072: free size of one (pt,ctp) w slab

_cache = {}


def _build_nc():
    import concourse.bacc as bacc
    import concourse.mybir as mybir
    from concourse import tile

    f32 = mybir.dt.float32
    bf16 = mybir.dt.bfloat16
    fp8 = mybir.dt.float8e4
    DR = mybir.MatmulPerfMode.DoubleRow
    OP = mybir.AluOpType
    AF = mybir.ActivationFunctionType

    nc = bacc.Bacc(trn_type="TRN2")

    u8_d = nc.dram_tensor("u8", [NPT, P, CTP * WSF], fp8,
                          kind="ExternalInput")
    xt8_d = nc.dram_tensor("xt8", [NPT * CTP, P, XCF], fp8,
                           kind="ExternalInput")
    cb_d = nc.dram_tensor("cb", [P, DT], f32, kind="ExternalInput")
    sw_d = nc.dram_tensor("sw", [P, 32], fp8, kind="ExternalInput")
    out_d = nc.dram_tensor("out", [1, PADO], f32, kind="ExternalOutput")

    with tile.TileContext(nc) as tc:
        with (
            tc.tile_pool(name="const", bufs=1) as cp,
            tc.tile_pool(name="wps_", bufs=6) as wp,
            tc.tile_pool(name="xps_", bufs=10) as xp,
            tc.tile_pool(name="rp", bufs=4) as rp,
            tc.tile_pool(name="cps", bufs=6, space="PSUM") as cps,
            tc.tile_pool(name="sps", bufs=1, space="PSUM") as sps,
        ):
            # PE warm-up first: memsets on GpSimd (free ~1us before Vector
            # in the framework preamble), then dummy bf16 matmuls with no
            # DMA deps run during the DMA lead-in, flipping the HAM clock
            # gate to 8/8 before the real conv stream starts.
            wrm_in = cp.tile([P, 2 * P], bf16)
            nc.gpsimd.memset(wrm_in[:], 0.0)
            ones_cb = cp.tile([P, 1], bf16)
            nc.gpsimd.memset(ones_cb[:], 1.0)
            for i in range(11):
                wrm_ps = cps.tile([1, 2 * P], f32, tag="cv", name=f"wrm{i}")
                nc.tensor.matmul(wrm_ps[:], ones_cb[:], wrm_in[:],
                                 start=True, stop=True)

            # first (pt0,ctp0) tiles: xt0 on the scalar ring runs in
            # parallel with the w chunks on the sync ring, so the first
            # conv matmul's data lands as the warm-up matmuls finish
            xt0 = xp.tile([P, XCF], fp8, name="xt", tag="xt")
            nc.scalar.dma_start(xt0[:], xt8_d[0])
            xt1 = xp.tile([P, XCF], fp8, name="xt", tag="xt")
            nc.scalar.dma_start(xt1[:], xt8_d[1])
            w_dts0 = []
            for dt in range(DT):
                w_dt = wp.tile([P, 6 * P], fp8, name="w_dt", tag="w_sl")
                nc.sync.dma_start(
                    w_dt[:], u8_d[0][:, dt * 6 * P:(dt + 1) * 6 * P])
                w_dts0.append(w_dt)

            # ---------- small constants (scalar HWDGE ring) ----------
            cb_sb = cp.tile([P, DT], f32)
            nc.scalar.dma_start(cb_sb[:], cb_d[:])
            # sw8[p, two*16 + g]: DR lhsT interleave stride must be %16
            sw_sb = cp.tile([P, 32], fp8)
            nc.scalar.dma_start(sw_sb[:], sw_d[:])

            # SBUF accumulators for the A^T fold, per d-tile
            o0 = [cp.tile([P, NF], f32, name=f"o0_{dt}") for dt in range(DT)]
            o1 = [cp.tile([P, NF3], f32, name=f"o1_{dt}")
                  for dt in range(DT)]
            relu_a = [cp.tile([P, 2 * NF], fp8, name=f"ra_{g}")
                      for g in range(2)]
            relu_b = [cp.tile([P, 2 * NF3], fp8, name=f"rb_{g}")
                      for g in range(2)]

            # score psums [1, 448] x2 accumulate logits across all dt
            s_ps = [sps.tile([1, NF if h == 0 else NF3], f32,
                             tag=f"s{h}", name=f"s_ps{h}")
                    for h in range(2)]

            # ---------- conv main: pt-outer passes, JIT streaming ------
            # pass 3 feeds only o1 (odd-x outputs); width-tile j=3's odd
            # output is the x=7 pad column, so pass 3 streams j 0..2 only.
            for pt in range(NPT):
                nfp = NF3 if pt == 3 else NF
                njp = 3 if pt == 3 else NJ
                xcf = 2 * T * YP * njp
                ps = [cps.tile([P, nfp], f32, tag="cv", name=f"ps{pt}_{dt}")
                      for dt in range(DT)]
                for ctp in range(CTP):
                    if pt == 3 and ctp == CTP - 1:
                        # score-A matmuls are data-ready (relu_a finished
                        # during this pass); slotting them here keeps them
                        # off the tail critical path
                        swv = sw_sb[:].rearrange(
                            "p (two x) -> p two x", two=2)
                        for g in range(2):
                            nc.tensor.matmul(
                                s_ps[0][:], swv[:, :, g:g + 1],
                                relu_a[g][:].rearrange(
                                    "p (two f) -> p two f", two=2),
                                start=(g == 0), stop=(g == 1),
                                perf_mode=DR)
                    w_dts = None
                    if pt == 0 and ctp == 0:
                        xt = xt0
                        w_sl = None
                        w_dts = w_dts0
                    else:
                        if pt == 0 and ctp == 1:
                            xt = xt1
                        else:
                            xt = xp.tile([P, xcf], fp8, name="xt",
                                         tag="xt")
                            nc.sync.dma_start(
                                xt[:], xt8_d[pt * CTP + ctp][:, 0:xcf])
                        w_sl = wp.tile([P, WSF], fp8, name="w_sl",
                                       tag="w_sl")
                        nc.sync.dma_start(
                            w_sl[:], u8_d[pt][:, ctp * WSF:(ctp + 1) * WSF])
                    xv = xt[:].rearrange("p (two t y j) -> p two t y j",
                                         two=2, t=T, y=YP)
                    dts = (list(reversed(range(DT)))
                           if pt == 3 and ctp == CTP - 1 else range(DT))
                    for dt in dts:
                        for dy in range(3):
                            if w_sl is None:
                                wsl2 = w_dts[dt][:, dy * 2 * P:
                                                 (dy + 1) * 2 * P]
                            else:
                                off = (dt * 3 + dy) * 2 * P
                                wsl2 = w_sl[:, off:off + 2 * P]
                            lhsT3 = wsl2.rearrange("p (two q) -> p two q",
                                                   two=2)
                            rhs = xv[:, :, :, dy:dy + H, :]
                            nc.tensor.matmul(
                                ps[dt][:], lhsT3, rhs,
                                start=(ctp == 0 and dy == 0),
                                stop=(ctp == CTP - 1 and dy == 2),
                                perf_mode=DR)
                # fold pass into o0/o1 (A^T signs); one PSUM operand per
                # op.  pt3: reversed to match the last group's matmul order
                for dt in (list(reversed(range(DT))) if pt == 3
                           else range(DT)):
                    if pt == 0:
                        nc.vector.tensor_copy(o0[dt][:], ps[dt][:])
                    elif pt == 1:
                        nc.vector.tensor_tensor(o0[dt][:], o0[dt][:],
                                                ps[dt][:], op=OP.add)
                        pv = ps[dt][:].rearrange(
                            "p (ty j) -> p ty j", j=NJ)[:, :, 0:3]
                        nc.vector.tensor_copy(o1[dt][:], pv)
                    elif pt == 2:
                        nc.vector.tensor_tensor(o0[dt][:], o0[dt][:],
                                                ps[dt][:], op=OP.add)
                        pv = ps[dt][:].rearrange(
                            "p (ty j) -> p ty j", j=NJ)[:, :, 0:3]
                        nc.vector.tensor_tensor(o1[dt][:], o1[dt][:],
                                                pv, op=OP.subtract)
                    else:
                        nc.vector.tensor_tensor(o1[dt][:], o1[dt][:],
                                                ps[dt][:], op=OP.subtract)
                if pt == 2:
                    # o0 is final after pass 2 -- its ReLUs (ACT) overlap
                    # pass 3's conv matmuls, so only the o1 side is tail.
                    for dt in range(DT):
                        dst = relu_a[dt // 2][:, (dt % 2) * NF:
                                              (dt % 2 + 1) * NF]
                        nc.scalar.activation(dst, o0[dt][:], AF.Relu,
                                             bias=cb_sb[:, dt:dt + 1],
                                             scale=1.0 / 64.0)

            # ---------- o1 ReLU + score + ship logits ----------
            xrow = cp.tile([1, PADO], f32)
            nc.vector.tensor_copy(xrow[0:1, 0:NF], s_ps[0][:])
            nc.sync.dma_start(out_d[:, 0:NF], xrow[0:1, 0:NF])
            swv2 = sw_sb[:].rearrange(
                "p (two x) -> p two x", two=2)
            for dt in (3, 2, 1, 0):
                dst = relu_b[dt // 2][:, (dt % 2) * NF3:(dt % 2 + 1) * NF3]
                nc.scalar.activation(dst, o1[dt][:], AF.Relu,
                                     bias=cb_sb[:, dt:dt + 1],
                                     scale=1.0 / 64.0)
                if dt % 2 == 0:
                    g = dt // 2
                    nc.tensor.matmul(
                        s_ps[1][:], swv2[:, :, g:g + 1],
                        relu_b[g][:].rearrange(
                            "p (two f) -> p two f", two=2),
                        start=(g == 1), stop=(g == 0),
                        perf_mode=DR)
            nc.vector.tensor_copy(xrow[0:1, NF:PADO], s_ps[1][:])
            nc.sync.dma_start(out_d[:, NF:PADO], xrow[0:1, NF:PADO])

    nc.compile()
    return nc


def _prep_in_maps(x, x_fpv_pred, proj_weight, conv1_w, conv1_b, score_w,
                  score_b):
    import concourse.mybir as mybir
    bf16 = ml_dtypes.bfloat16
    fp8 = mybir.dt.np(mybir.dt.float8e4)

    # x_tilde: pad (y: 1+7+1, x: 1+7+2), B^T along x per 2-wide tile
    xr = np.asarray(x, np.float32).reshape(B, CT, P, T, H, W)
    xpad = np.zeros((B, CT, P, T, YP, 10), np.float32)
    xpad[:, :, :, :, 1:8, 1:8] = xr
    # windows d[k] = xpad[..., 2j+k], k=0..3, j=0..3
    dw = np.stack([xpad[..., 2 * j:2 * j + 4] for j in range(NJ)], axis=-2)
    # dw: [B, CT, P, T, YP, NJ, 4]
    xt = np.empty((B, NPT, CT, P, T, YP, NJ), np.float32)
    xt[:, 0] = dw[..., 0] - dw[..., 2]
    xt[:, 1] = dw[..., 1] + dw[..., 2]
    xt[:, 2] = dw[..., 2] - dw[..., 1]
    xt[:, 3] = dw[..., 1] - dw[..., 3]
    # device layout [B, (pt, ctp), P, (two, t, y, j)]; pt3 packs j 0..2
    xt = xt.reshape(B, NPT, CTP, 2, P, T, YP, NJ)
    xtt = xt.transpose(0, 1, 2, 4, 3, 5, 6, 7)  # [B,pt,ctp,P,two,T,YP,NJ]
    xt8 = np.zeros((B, NPT * CTP, P, XCF), fp8)
    xt8[:, :3 * CTP] = xtt[:, :3].reshape(B, 3 * CTP, P, XCF).astype(fp8)
    xt8[:, 3 * CTP:, :, :2 * T * YP * 3] = np.ascontiguousarray(
        xtt[:, 3, :, :, :, :, :, 0:3]).reshape(
        B, CTP, P, 2 * T * YP * 3).astype(fp8)

    # U = G @ w along dx, x64, fp8.  G rows: [1,0,0],[.5,.5,.5],
    # [.5,-.5,.5],[0,0,1]
    w9 = np.asarray(conv1_w, np.float32).reshape(D, C, 3, 3)
    G = np.array([[1, 0, 0], [.5, .5, .5], [.5, -.5, .5], [0, 0, 1]],
                 np.float32)
    u = np.einsum('pk,dcyk->dcyp', G, w9) * 64.0   # [D, C, 3dy, NPT]
    # layout u8[pt, p, ((ctp*DT + dt)*3 + dy)*2*P + two*P + q]
    #   = u[dt*P+q, (2*ctp+two)*P+p, dy, pt]
    u8 = np.ascontiguousarray(
        u.reshape(DT, P, CTP, 2, P, 3, NPT)
        .transpose(6, 4, 2, 0, 5, 3, 1).reshape(NPT, P, CTP * WSF)
    ).astype(fp8)

    cb = np.ascontiguousarray(
        np.asarray(conv1_b, np.float32).reshape(DT, P).T)
    # sw8[p, two*16 + g] = 64 * score_w[(2g+two)*P + p] (stride-16 pairs)
    swr = (np.asarray(score_w, np.float32).reshape(2, 2, P) * 64.0)
    sw = np.zeros((P, 32), np.float32)
    for g in range(2):
        for two in range(2):
            sw[:, two * 16 + g] = swr[g, two]
    sw = np.ascontiguousarray(sw).astype(fp8)

    in_maps = []
    for b in range(B):
        in_maps.append({
            "xt8": xt8[b],
            "u8": u8,
            "cb": cb,
            "sw": sw,
        })
    return in_maps


def _host_cam_target(x, x_fpv_pred, proj_weight):
    """Exact fp32 CAM target (reference semantics), vectorized numpy."""
    xf = np.asarray(x, np.float32).reshape(B, C, -1)
    top_cls = np.argmax(np.asarray(x_fpv_pred, np.float32), axis=1)
    rows = np.asarray(proj_weight, np.float32)[top_cls]       # [B, C]
    cam = np.einsum('bc,bcn->bn', rows, xf)                   # [B, 784]
    cmin = cam.min(axis=1, keepdims=True)
    cmax = cam.max(axis=1, keepdims=True)
    cam_n = (cam - cmin) / (cmax - cmin)
    y = np.zeros_like(cam_n)
    idx = np.argpartition(-cam_n, N_TOKEN - 1, axis=1)[:, :N_TOKEN]
    np.put_along_axis(y, idx, np.take_along_axis(cam_n, idx, axis=1), axis=1)
    return y                                                   # [B, 784]


def run(inputs, trace=False):
    """Build (cached), run on 8 cores, return (loss, BassKernelResults)."""
    from concourse.bass_utils import run_bass_kernel_spmd

    if "nc" not in _cache:
        _cache["nc"] = _build_nc()
    nc = _cache["nc"]
    in_maps = _prep_in_maps(**inputs)
    res = run_bass_kernel_spmd(nc, in_maps, core_ids=list(range(B)),
                               trace=trace)
    y = _host_cam_target(inputs["x"], inputs["x_fpv_pred"],
                         inputs["proj_weight"])
    sb = float(np.asarray(inputs["score_b"], np.float32).reshape(-1)[0])
    total = 0.0
    for b in range(B):
        row = np.asarray(res.results[b]["out"], np.float32).reshape(PADO)
        x8 = np.empty((T * H, W), np.float32)                  # x = 2j+k
        x8[:, 0::2] = row[:NF].reshape(T * H, NJ)              # x 0,2,4,6
        x8[:, 1::2] = row[NF:].reshape(T * H, 3)               # x 1,3,5
        xcam = x8.reshape(-1) / 64.0 + sb
        # bce_sum = sum log(1+e^x) - sum x*y  (softplus stable form)
        sp = np.logaddexp(0.0, xcam).sum()
        total += float(sp - (xcam * y[b]).sum())
    loss = np.float32(total / float(B * T * H * W))
    return loss, res


def kernel(**inputs):
    loss, _ = run(inputs, trace=False)
    return loss
